# revision 38
# baseline (speedup 1.0000x reference)
"""Trainium2 Bass kernel for nn_NumAttention (sparse_attention).

Reference computation (per batch b, head i):
    k     = blockmix(x_cat, softmax(W_K)[i])            # [P, DH]
    xq    = blockmix(x_cat, softmax(W_Q)[i])            # [P, DH]
    q     = xq @ softmax(W_pred)[i]                     # [P, DH]
    v     = x_num @ softmax(W_V)[i]                     # [P]
    z[qp] = sum_{p<=qp} v[p] * (k[p] . q[qp])           # causal, no softmax

Softmax-free scalar-value attention is linear: z[qp] = xq[qp] . S[qp] with
S = cumsum_p(v[p] * ktilde[p,:]), ktilde = k @ pp^T (W_pred folded into the
k side).  No O(P^2) score matrix.

This version runs the whole computation as ONE software-pipelined stream of
chunk pairs so the PE never idles (idling drops the HAM clock to half rate
and strands the vector-engine epilogue as a serial tail).  Per pair j,
lagging the mix by one pair:

  S = trit @ vk_pair                      (cumsum; positions chunk-reversed)
    + ones_sq @ texw[j]                   (carry-A: Tex[2j] broadcast; texw
                                           row 0 = Tex, rows 1-127 = 0)
    + ones_sq @ vk[2j] -> right half only (carry-B: +T[2j])
  then ACT drains S to bf16 and DVE does prod+reduce into z.

The chunk-reversal puts each chunk's full column sum on PSUM partition 0,
so S[0, right] == Tex[2j+2] directly: one ACT broadcast-copy of that row
stages the next pair's texw.  No prefix accumulation pass exists at all.

Sharding: 8 cores = 4 batches x 2 head-groups (4 heads each).  Host ships
x_cat[b] pre-transposed feature-major bf16 (4KB DMA lines), the folded
per-head weight matrices, and host-computed v.  Early wide dummy matmuls
warm the PE clock while the first DMA slices land.
"""

import numpy as np
import ml_dtypes

import concourse.bacc as bacc
import concourse.mybir as mybir
import concourse.tile as tile
from concourse.bass_utils import run_bass_kernel_spmd

B, P, DC, DN, H, DH = 4, 2048, 512, 64, 8, 64
NV = DC // DH
CH = 128          # positions per chunk
NCH = P // CH     # 16 chunks
NPR = NCH // 2    # 8 chunk pairs
HPC = 4           # heads per core
FH = HPC * DH     # 256 = stacked-head free width
FH2 = 2 * FH      # 512 = pair width
NCORES = 8
KC = DC // CH     # 4 feature K-chunks
NWARM = 7         # PE warm-up dummy matmuls

_BF16 = ml_dtypes.bfloat16

_cache = {}


def _softmax(x, axis=-1):
    e = np.exp(x - x.max(axis=axis, keepdims=True))
    return e / e.sum(axis=axis, keepdims=True)


def _build_program():
    nc = bacc.Bacc()
    f32 = mybir.dt.float32
    bf16 = mybir.dt.bfloat16
    mult = mybir.AluOpType.mult
    add = mybir.AluOpType.add

    w_d = nc.dram_tensor("w", [CH, KC, FH2], bf16, kind="ExternalInput")
    xct_d = nc.dram_tensor("xct", [4, CH, KC, P // 4], bf16, kind="ExternalInput")
    v_d = nc.dram_tensor("v", [CH, NCH * HPC], f32, kind="ExternalInput")
    trit_d = nc.dram_tensor("trit", [CH, CH], bf16, kind="ExternalInput")
    z_d = nc.dram_tensor("z", [CH, NCH * HPC], f32, kind="ExternalOutput")

    with tile.TileContext(nc) as tc:
        with (
            tc.tile_pool(name="persist", bufs=1) as pers,
            tc.tile_pool(name="work", bufs=3) as work,
            tc.tile_pool(name="mixp", bufs=3, space="PSUM") as mixp,
            tc.tile_pool(name="sp", bufs=2, space="PSUM") as sp,
            tc.tile_pool(name="psmall", bufs=1, space="PSUM") as psmall,
        ):
            w_sb = pers.tile([CH, KC, FH2], bf16, tag="w_sb")
            xcT = pers.tile([CH, 4, KC, P // 4], bf16, tag="xcT")
            v_sb = pers.tile([CH, NCH * HPC], f32, tag="v_sb")
            trit_sb = pers.tile([CH, CH], bf16, tag="trit_sb")
            vk_sb = pers.tile([CH, NCH, FH], bf16, tag="vk_sb")
            q_sb = pers.tile([CH, NCH, FH], bf16, tag="q_sb")
            z_sb = pers.tile([CH, NCH * HPC], f32, tag="z_sb")
            dumw = pers.tile([CH, FH2], bf16, tag="dumw")
            ones_sq = pers.tile([CH, CH], bf16, tag="ones_sq")
            # texw ping-pong: row 0 carries Tex[2j], rows 1-127 stay zero so
            # the all-ones stationary broadcasts row 0 (no K=1 matmul)
            texw_pp = [
                pers.tile([CH, FH2], bf16, name="texw0", tag="texw0"),
                pers.tile([CH, FH2], bf16, name="texw1", tag="texw1"),
            ]

            # ---- memsets + PE warm-up: wide dummy matmuls release the HAM
            # clock throttle while the DMA head is still in flight
            nc.gpsimd.memset(dumw[:], 0.0)
            nc.gpsimd.memset(ones_sq[:], 1.0)
            nc.gpsimd.memset(texw_pp[0][:], 0.0)
            nc.gpsimd.memset(texw_pp[1][:], 0.0)
            psum_dum = psmall.tile([CH, FH2], f32, tag="psum_dum")
            for i in range(NWARM):
                nc.tensor.matmul(
                    psum_dum[:], dumw[:, 0:CH], dumw[:], start=True, stop=True
                )

            # ---- loads: two HWDGE rings, 4KB per-partition lines.  Slice s
            # covers chunks {4s..4s+3} (pairs 2s, 2s+1).  sync ring: w (one
            # 512KB burst), v, trit, slices 1,3; scalar ring: slices 0,2.
            nc.sync.dma_start(out=w_sb[:], in_=w_d[:])
            nc.scalar.dma_start(out=xcT[:, 0], in_=xct_d[0])
            nc.sync.dma_start(out=v_sb[:], in_=v_d[:])
            nc.sync.dma_start(out=trit_sb[:], in_=trit_d[:])
            nc.sync.dma_start(out=xcT[:, 1], in_=xct_d[1])
            nc.scalar.dma_start(out=xcT[:, 2], in_=xct_d[2])
            nc.sync.dma_start(out=xcT[:, 3], in_=xct_d[3])


            def emit_mix(c):
                psum_mix = mixp.tile([CH, FH2], f32, tag="psum_mix")
                for kc in range(KC):
                    nc.tensor.matmul(
                        psum_mix[:],
                        xcT[:, c // 4, kc, (c % 4) * CH : (c % 4 + 1) * CH],
                        w_sb[:, kc, :],
                        start=(kc == 0),
                        stop=(kc == KC - 1),
                    )
                # vk[p, i, h] = ktilde[p, i, h] * v[p, i]
                nc.vector.tensor_tensor(
                    out=vk_sb[:, c, :].rearrange("p (i h) -> p i h", h=DH),
                    in0=psum_mix[:, 0:FH].rearrange("p (i h) -> p i h", h=DH),
                    in1=v_sb[:, c * HPC : (c + 1) * HPC]
                    .unsqueeze(2)
                    .broadcast_to([CH, HPC, DH]),
                    op=mult,
                )
                nc.scalar.copy(q_sb[:, c, :], psum_mix[:, FH:FH2])

            def emit_pass2_mm(j):
                """S = trit @ vk_pair + carry (see module docstring).

                Pair 0 skips carry-A since Tex[0] = 0.
                """
                psum_S = sp.tile([CH, FH2], f32, tag="psum_S")
                nc.tensor.matmul(
                    psum_S[:],
                    trit_sb[:],
                    vk_sb[:, 2 * j : 2 * j + 2, :].rearrange("p c f -> p (c f)"),
                    start=True,
                    stop=False,
                )
                nc.tensor.matmul(
                    psum_S[:, FH:FH2],
                    ones_sq[:],
                    vk_sb[:, 2 * j, :],
                    start=False,
                    stop=(j == 0),
                )
                if j >= 1:
                    nc.tensor.matmul(
                        psum_S[:], ones_sq[:], texw_pp[j % 2][:], start=False,
                        stop=True,
                    )
                if j < NPR - 1:
                    # positions are chunk-reversed, so the full cumsum (the
                    # next pair's Tex) sits on partition 0 of the right half
                    nc.scalar.copy(
                        texw_pp[(j + 1) % 2][0:1, :].rearrange(
                            "a (c f) -> a c f", f=FH
                        ),
                        psum_S[0:1, FH:FH2].unsqueeze(1).broadcast_to([1, 2, FH]),
                    )
                return psum_S

            def emit_pass2_ve(j, psum_S, split=1):
                """ACT drain + DVE prod/reduce for pair j.

                split=2 pipelines the chain in half-pair chunks (used for the
                last pair, whose chain is the exposed tail).
                """
                for h in range(split):
                    f0, f1 = h * FH2 // split, (h + 1) * FH2 // split
                    s_sb = work.tile([CH, FH2 // split], bf16, tag=f"s_sb{split}")
                    nc.scalar.copy(s_sb[:], psum_S[:, f0:f1])
                    prod = work.tile([CH, FH2 // split], bf16, tag=f"prod{split}")
                    nc.vector.tensor_tensor(
                        out=prod[:],
                        in0=q_sb[:, 2 * j : 2 * j + 2, :].rearrange(
                            "p c f -> p (c f)"
                        )[:, f0:f1],
                        in1=s_sb[:],
                        op=mult,
                    )
                    nc.vector.tensor_reduce(
                        out=z_sb[
                            :,
                            2 * j * HPC + h * 2 * HPC // split : 2 * j * HPC
                            + (h + 1) * 2 * HPC // split,
                        ],
                        in_=prod[:].rearrange("p (ci h) -> p ci h", h=DH),
                        axis=mybir.AxisListType.X,
                        op=add,
                    )

            # ---- software-pipelined main stream, pass-2 lagging one pair
            prev_S = None
            for j in range(NPR):
                emit_mix(2 * j)
                if j >= 1:
                    prev_S = emit_pass2_mm(j - 1)
                emit_mix(2 * j + 1)
                if j >= 1:
                    emit_pass2_ve(j - 1, prev_S)
                if j == NPR - 1:
                    # overlap most of the z write-back with the last pair
                    nc.sync.dma_start(
                        out=z_d[:, 0 : 14 * HPC], in_=z_sb[:, 0 : 14 * HPC]
                    )
            prev_S = emit_pass2_mm(NPR - 1)
            # keep the PE (and its clock) busy while the last ve-chain drains
            for i in range(6):
                nc.tensor.matmul(
                    psum_dum[:], dumw[:, 0:CH], dumw[:], start=True, stop=True
                )
            emit_pass2_ve(NPR - 1, prev_S, split=2)

            nc.sync.dma_start(
                out=z_d[:, 14 * HPC : NCH * HPC], in_=z_sb[:, 14 * HPC : NCH * HPC]
            )

    nc.finalize()
    return nc


def _host_inputs(x_cat, x_num, W_K, W_Q, W_pred, W_V):
    """Per-core input maps. Core c = batch (c//2), head-group (c%2)."""
    pk = _softmax(W_K.astype(np.float64)).astype(np.float32)
    pq = _softmax(W_Q.astype(np.float64)).astype(np.float32)
    pp = _softmax(W_pred.astype(np.float64)).astype(np.float32)
    pv = _softmax(W_V.astype(np.float64)).astype(np.float32)

    # positions are stored chunk-reversed (r = 127 - p%128) so the inclusive
    # cumsum matmul leaves the full chunk sum on partition 0
    trit = np.tril(np.ones((CH, CH), np.float32))
    perm = np.arange(P).reshape(NCH, CH)[:, ::-1].reshape(P)
    eye = np.eye(DH, dtype=np.float32)
    v_full = np.einsum("bpd,id->bpi", x_num, pv)  # [B, P, H] fp32, host-side

    in_maps = []
    for core in range(NCORES):
        b, hg = core // 2, core % 2
        heads = range(hg * HPC, (hg + 1) * HPC)
        W = np.zeros((DC, FH2), np.float32)
        for j, i in enumerate(heads):
            # ktilde cols: W[(v,g), j*64+h] = pk[i,v] * pp[i,h,g]
            W[:, j * DH : (j + 1) * DH] = (
                pk[i][:, None, None] * pp[i].T[None, :, :]
            ).reshape(DC, DH)
            # xq cols: W[(v,h), FH + j*64+h'] = pq[i,v] * delta(h,h')
            W[:, FH + j * DH : FH + (j + 1) * DH] = np.kron(pq[i][:, None], eye)
        # per-partition contiguous slice blocks (4KB lines), chunk-reversed
        xq8 = x_cat[b][perm].T.reshape(KC, CH, 4, P // 4).transpose(2, 1, 0, 3)
        wq = W.reshape(KC, CH, FH2).transpose(1, 0, 2)
        # v in device layout [p, (chunk, head)]
        v_core = v_full[b][perm][:, hg * HPC : (hg + 1) * HPC]  # [P, HPC]
        v_dev = np.ascontiguousarray(
            v_core.reshape(NCH, CH, HPC).transpose(1, 0, 2).reshape(CH, NCH * HPC)
        )
        in_maps.append(
            {
                "xct": np.ascontiguousarray(xq8).astype(_BF16),
                "w": np.ascontiguousarray(wq).astype(_BF16),
                "v": v_dev,
                "trit": trit.astype(_BF16),
            }
        )
    return in_maps


def _run(inputs, **spmd_kwargs):
    if "nc" not in _cache:
        _cache["nc"] = _build_program()
    nc = _cache["nc"]

    in_maps = _host_inputs(**inputs)
    res = run_bass_kernel_spmd(nc, in_maps, list(range(NCORES)), **spmd_kwargs)

    perm = np.arange(P).reshape(NCH, CH)[:, ::-1].reshape(P)
    out = np.zeros((B, P, H), np.float32)
    for core in range(NCORES):
        b, hg = core // 2, core % 2
        z = res.results[core]["z"]  # [128, NCH*HPC]
        z = z.reshape(CH, NCH, HPC).transpose(1, 0, 2).reshape(P, HPC)
        out[b, :, hg * HPC : (hg + 1) * HPC] = z[perm]
    return out, res


def kernel(x_cat, x_num, W_K, W_Q, W_pred, W_V):
    out, _ = _run(
        dict(x_cat=x_cat, x_num=x_num, W_K=W_K, W_Q=W_Q, W_pred=W_pred, W_V=W_V)
    )
    return out


# revision 40
# speedup vs baseline: 1.0311x; 1.0311x over previous
"""Trainium2 Bass kernel for nn_NumAttention (sparse_attention).

Reference computation (per batch b, head i):
    k     = blockmix(x_cat, softmax(W_K)[i])            # [P, DH]
    xq    = blockmix(x_cat, softmax(W_Q)[i])            # [P, DH]
    q     = xq @ softmax(W_pred)[i]                     # [P, DH]
    v     = x_num @ softmax(W_V)[i]                     # [P]
    z[qp] = sum_{p<=qp} v[p] * (k[p] . q[qp])           # causal, no softmax

Softmax-free scalar-value attention is linear: z[qp] = xq[qp] . S[qp] with
S = cumsum_p(v[p] * ktilde[p,:]), ktilde = k @ pp^T (W_pred folded into the
k side).  No O(P^2) score matrix.

This version runs the whole computation as ONE software-pipelined stream of
chunk pairs so the PE never idles (idling drops the HAM clock to half rate
and strands the vector-engine epilogue as a serial tail).  Per pair j,
lagging the mix by one pair:

  S = trit @ vk_pair                      (cumsum; positions chunk-reversed)
    + ones_sq @ texw[j]                   (carry-A: Tex[2j] broadcast; texw
                                           row 0 = Tex, rows 1-127 = 0)
    + ones_sq @ vk[2j] -> right half only (carry-B: +T[2j])
  then ACT drains S to bf16 and DVE does prod+reduce into z.

The chunk-reversal puts each chunk's full column sum on PSUM partition 0,
so S[0, right] == Tex[2j+2] directly: one ACT broadcast-copy of that row
stages the next pair's texw.  No prefix accumulation pass exists at all.

Sharding: 8 cores = 4 batches x 2 head-groups (4 heads each).  Host ships
x_cat[b] pre-transposed feature-major bf16 (4KB DMA lines), the folded
per-head weight matrices, and host-computed v.  Early wide dummy matmuls
warm the PE clock while the first DMA slices land.
"""

import numpy as np
import ml_dtypes

import concourse.bacc as bacc
import concourse.mybir as mybir
import concourse.tile as tile
from concourse.bass_utils import run_bass_kernel_spmd

B, P, DC, DN, H, DH = 4, 2048, 512, 64, 8, 64
NV = DC // DH
CH = 128          # positions per chunk
NCH = P // CH     # 16 chunks
NPR = NCH // 2    # 8 chunk pairs
HPC = 4           # heads per core
FH = HPC * DH     # 256 = stacked-head free width
FH2 = 2 * FH      # 512 = pair width
NCORES = 8
KC = DC // CH     # 4 feature K-chunks
NWARM = 9         # PE warm-up dummy matmuls

_BF16 = ml_dtypes.bfloat16

_cache = {}


def _softmax(x, axis=-1):
    e = np.exp(x - x.max(axis=axis, keepdims=True))
    return e / e.sum(axis=axis, keepdims=True)


def _build_program():
    nc = bacc.Bacc()
    f32 = mybir.dt.float32
    bf16 = mybir.dt.bfloat16
    mult = mybir.AluOpType.mult
    add = mybir.AluOpType.add

    w_d = nc.dram_tensor("w", [CH, KC, FH2], bf16, kind="ExternalInput")
    xct_d = nc.dram_tensor("xct", [4, CH, KC, P // 4], bf16, kind="ExternalInput")
    v_d = nc.dram_tensor("v", [CH, NCH * HPC], f32, kind="ExternalInput")
    trit_d = nc.dram_tensor("trit", [CH, CH], bf16, kind="ExternalInput")
    z_d = nc.dram_tensor("z", [CH, NCH * HPC], f32, kind="ExternalOutput")

    with tile.TileContext(nc) as tc:
        with (
            tc.tile_pool(name="persist", bufs=1) as pers,
            tc.tile_pool(name="work", bufs=3) as work,
            tc.tile_pool(name="mixp", bufs=3, space="PSUM") as mixp,
            tc.tile_pool(name="sp", bufs=2, space="PSUM") as sp,
            tc.tile_pool(name="psmall", bufs=1, space="PSUM") as psmall,
        ):
            w_sb = pers.tile([CH, KC, FH2], bf16, tag="w_sb")
            xcT = pers.tile([CH, 4, KC, P // 4], bf16, tag="xcT")
            v_sb = pers.tile([CH, NCH * HPC], f32, tag="v_sb")
            trit_sb = pers.tile([CH, CH], bf16, tag="trit_sb")
            vk_sb = pers.tile([CH, NCH, FH], bf16, tag="vk_sb")
            q_sb = pers.tile([CH, NCH, FH], bf16, tag="q_sb")
            z_sb = pers.tile([CH, NCH * HPC], f32, tag="z_sb")
            dumw = pers.tile([CH, FH2], bf16, tag="dumw")
            ones_sq = pers.tile([CH, CH], bf16, tag="ones_sq")
            # texw ping-pong: row 0 carries Tex[2j], rows 1-127 stay zero so
            # the all-ones stationary broadcasts row 0 (no K=1 matmul)
            texw_pp = [
                pers.tile([CH, FH2], bf16, name="texw0", tag="texw0"),
                pers.tile([CH, FH2], bf16, name="texw1", tag="texw1"),
            ]

            # ---- memsets + PE warm-up: wide dummy matmuls release the HAM
            # clock throttle while the DMA head is still in flight
            nc.gpsimd.memset(dumw[:], 0.0)
            nc.gpsimd.memset(ones_sq[:], 1.0)
            nc.gpsimd.memset(texw_pp[0][:], 0.0)
            nc.gpsimd.memset(texw_pp[1][:], 0.0)
            psum_dum = psmall.tile([CH, FH2], f32, tag="psum_dum")
            for i in range(NWARM):
                nc.tensor.matmul(
                    psum_dum[:], dumw[:, 0:CH], dumw[:], start=True, stop=True
                )

            # ---- loads: two HWDGE rings, 4KB per-partition lines.  Slice s
            # covers chunks {4s..4s+3} (pairs 2s, 2s+1).  sync ring: w (one
            # 512KB burst), v, trit, slices 1,3; scalar ring: slices 0,2.
            nc.scalar.dma_start(out=w_sb[:], in_=w_d[:])
            nc.sync.dma_start(out=xcT[:, 0], in_=xct_d[0])
            nc.sync.dma_start(out=v_sb[:], in_=v_d[:])
            nc.sync.dma_start(out=trit_sb[:], in_=trit_d[:])
            nc.scalar.dma_start(out=xcT[:, 1], in_=xct_d[1])
            nc.sync.dma_start(out=xcT[:, 2], in_=xct_d[2])
            nc.scalar.dma_start(out=xcT[:, 3], in_=xct_d[3])


            def emit_mix(c):
                psum_mix = mixp.tile([CH, FH2], f32, tag="psum_mix")
                for kc in range(KC):
                    nc.tensor.matmul(
                        psum_mix[:],
                        xcT[:, c // 4, kc, (c % 4) * CH : (c % 4 + 1) * CH],
                        w_sb[:, kc, :],
                        start=(kc == 0),
                        stop=(kc == KC - 1),
                    )
                # vk[p, i, h] = ktilde[p, i, h] * v[p, i]
                nc.vector.tensor_tensor(
                    out=vk_sb[:, c, :].rearrange("p (i h) -> p i h", h=DH),
                    in0=psum_mix[:, 0:FH].rearrange("p (i h) -> p i h", h=DH),
                    in1=v_sb[:, c * HPC : (c + 1) * HPC]
                    .unsqueeze(2)
                    .broadcast_to([CH, HPC, DH]),
                    op=mult,
                )
                nc.scalar.copy(q_sb[:, c, :], psum_mix[:, FH:FH2])

            def emit_pass2_mm(j):
                """S = trit @ vk_pair + carry (see module docstring).

                Pair 0 skips carry-A since Tex[0] = 0.
                """
                psum_S = sp.tile([CH, FH2], f32, tag="psum_S")
                nc.tensor.matmul(
                    psum_S[:],
                    trit_sb[:],
                    vk_sb[:, 2 * j : 2 * j + 2, :].rearrange("p c f -> p (c f)"),
                    start=True,
                    stop=False,
                )
                nc.tensor.matmul(
                    psum_S[:, FH:FH2],
                    ones_sq[:],
                    vk_sb[:, 2 * j, :],
                    start=False,
                    stop=(j == 0),
                )
                if j >= 1:
                    nc.tensor.matmul(
                        psum_S[:], ones_sq[:], texw_pp[j % 2][:], start=False,
                        stop=True,
                    )
                if j < NPR - 1:
                    # positions are chunk-reversed, so the full cumsum (the
                    # next pair's Tex) sits on partition 0 of the right half
                    nc.scalar.copy(
                        texw_pp[(j + 1) % 2][0:1, :].rearrange(
                            "a (c f) -> a c f", f=FH
                        ),
                        psum_S[0:1, FH:FH2].unsqueeze(1).broadcast_to([1, 2, FH]),
                    )
                return psum_S

            def emit_pass2_ve(j, psum_S, split=1):
                """ACT drain + DVE prod/reduce for pair j.

                split=2 pipelines the chain in half-pair chunks (used for the
                last pair, whose chain is the exposed tail).
                """
                for h in range(split):
                    f0, f1 = h * FH2 // split, (h + 1) * FH2 // split
                    s_sb = work.tile([CH, FH2 // split], bf16, tag=f"s_sb{split}")
                    nc.scalar.copy(s_sb[:], psum_S[:, f0:f1])
                    prod = work.tile([CH, FH2 // split], bf16, tag=f"prod{split}")
                    nc.vector.tensor_tensor(
                        out=prod[:],
                        in0=q_sb[:, 2 * j : 2 * j + 2, :].rearrange(
                            "p c f -> p (c f)"
                        )[:, f0:f1],
                        in1=s_sb[:],
                        op=mult,
                    )
                    nc.vector.tensor_reduce(
                        out=z_sb[
                            :,
                            2 * j * HPC + h * 2 * HPC // split : 2 * j * HPC
                            + (h + 1) * 2 * HPC // split,
                        ],
                        in_=prod[:].rearrange("p (ci h) -> p ci h", h=DH),
                        axis=mybir.AxisListType.X,
                        op=add,
                    )

            # ---- software-pipelined main stream, pass-2 lagging one pair
            prev_S = None
            for j in range(NPR):
                emit_mix(2 * j)
                if j >= 1:
                    prev_S = emit_pass2_mm(j - 1)
                emit_mix(2 * j + 1)
                if j >= 1:
                    emit_pass2_ve(j - 1, prev_S)
                if j == NPR - 1:
                    # overlap most of the z write-back with the last pair
                    nc.sync.dma_start(
                        out=z_d[:, 0 : 14 * HPC], in_=z_sb[:, 0 : 14 * HPC]
                    )
            prev_S = emit_pass2_mm(NPR - 1)
            # keep the PE (and its clock) busy while the last ve-chain drains
            for i in range(6):
                nc.tensor.matmul(
                    psum_dum[:], dumw[:, 0:CH], dumw[:], start=True, stop=True
                )
            emit_pass2_ve(NPR - 1, prev_S, split=2)

            nc.sync.dma_start(
                out=z_d[:, 14 * HPC : NCH * HPC], in_=z_sb[:, 14 * HPC : NCH * HPC]
            )

    nc.finalize()
    return nc


def _host_inputs(x_cat, x_num, W_K, W_Q, W_pred, W_V):
    """Per-core input maps. Core c = batch (c//2), head-group (c%2)."""
    pk = _softmax(W_K.astype(np.float64)).astype(np.float32)
    pq = _softmax(W_Q.astype(np.float64)).astype(np.float32)
    pp = _softmax(W_pred.astype(np.float64)).astype(np.float32)
    pv = _softmax(W_V.astype(np.float64)).astype(np.float32)

    # positions are stored chunk-reversed (r = 127 - p%128) so the inclusive
    # cumsum matmul leaves the full chunk sum on partition 0
    trit = np.tril(np.ones((CH, CH), np.float32))
    perm = np.arange(P).reshape(NCH, CH)[:, ::-1].reshape(P)
    eye = np.eye(DH, dtype=np.float32)
    v_full = np.einsum("bpd,id->bpi", x_num, pv)  # [B, P, H] fp32, host-side

    in_maps = []
    for core in range(NCORES):
        b, hg = core // 2, core % 2
        heads = range(hg * HPC, (hg + 1) * HPC)
        W = np.zeros((DC, FH2), np.float32)
        for j, i in enumerate(heads):
            # ktilde cols: W[(v,g), j*64+h] = pk[i,v] * pp[i,h,g]
            W[:, j * DH : (j + 1) * DH] = (
                pk[i][:, None, None] * pp[i].T[None, :, :]
            ).reshape(DC, DH)
            # xq cols: W[(v,h), FH + j*64+h'] = pq[i,v] * delta(h,h')
            W[:, FH + j * DH : FH + (j + 1) * DH] = np.kron(pq[i][:, None], eye)
        # per-partition contiguous slice blocks (4KB lines), chunk-reversed
        xq8 = x_cat[b][perm].T.reshape(KC, CH, 4, P // 4).transpose(2, 1, 0, 3)
        wq = W.reshape(KC, CH, FH2).transpose(1, 0, 2)
        # v in device layout [p, (chunk, head)]
        v_core = v_full[b][perm][:, hg * HPC : (hg + 1) * HPC]  # [P, HPC]
        v_dev = np.ascontiguousarray(
            v_core.reshape(NCH, CH, HPC).transpose(1, 0, 2).reshape(CH, NCH * HPC)
        )
        in_maps.append(
            {
                "xct": np.ascontiguousarray(xq8).astype(_BF16),
                "w": np.ascontiguousarray(wq).astype(_BF16),
                "v": v_dev,
                "trit": trit.astype(_BF16),
            }
        )
    return in_maps


def _run(inputs, **spmd_kwargs):
    if "nc" not in _cache:
        _cache["nc"] = _build_program()
    nc = _cache["nc"]

    in_maps = _host_inputs(**inputs)
    res = run_bass_kernel_spmd(nc, in_maps, list(range(NCORES)), **spmd_kwargs)

    perm = np.arange(P).reshape(NCH, CH)[:, ::-1].reshape(P)
    out = np.zeros((B, P, H), np.float32)
    for core in range(NCORES):
        b, hg = core // 2, core % 2
        z = res.results[core]["z"]  # [128, NCH*HPC]
        z = z.reshape(CH, NCH, HPC).transpose(1, 0, 2).reshape(P, HPC)
        out[b, :, hg * HPC : (hg + 1) * HPC] = z[perm]
    return out, res


def kernel(x_cat, x_num, W_K, W_Q, W_pred, W_V):
    out, _ = _run(
        dict(x_cat=x_cat, x_num=x_num, W_K=W_K, W_Q=W_Q, W_pred=W_pred, W_V=W_V)
    )
    return out


# revision 41
# speedup vs baseline: 1.0399x; 1.0085x over previous
"""Trainium2 Bass kernel for nn_NumAttention (sparse_attention).

Reference computation (per batch b, head i):
    k     = blockmix(x_cat, softmax(W_K)[i])            # [P, DH]
    xq    = blockmix(x_cat, softmax(W_Q)[i])            # [P, DH]
    q     = xq @ softmax(W_pred)[i]                     # [P, DH]
    v     = x_num @ softmax(W_V)[i]                     # [P]
    z[qp] = sum_{p<=qp} v[p] * (k[p] . q[qp])           # causal, no softmax

Softmax-free scalar-value attention is linear: z[qp] = xq[qp] . S[qp] with
S = cumsum_p(v[p] * ktilde[p,:]), ktilde = k @ pp^T (W_pred folded into the
k side).  No O(P^2) score matrix.

This version runs the whole computation as ONE software-pipelined stream of
chunk pairs so the PE never idles (idling drops the HAM clock to half rate
and strands the vector-engine epilogue as a serial tail).  Per pair j,
lagging the mix by one pair:

  S = trit @ vk_pair                      (cumsum; positions chunk-reversed)
    + ones_sq @ texw[j]                   (carry-A: Tex[2j] broadcast; texw
                                           row 0 = Tex, rows 1-127 = 0)
    + ones_sq @ vk[2j] -> right half only (carry-B: +T[2j])
  then ACT drains S to bf16 and DVE does prod+reduce into z.

The chunk-reversal puts each chunk's full column sum on PSUM partition 0,
so S[0, right] == Tex[2j+2] directly: one ACT broadcast-copy of that row
stages the next pair's texw.  No prefix accumulation pass exists at all.

Sharding: 8 cores = 4 batches x 2 head-groups (4 heads each).  Host ships
x_cat[b] pre-transposed feature-major bf16 (4KB DMA lines), the folded
per-head weight matrices, and host-computed v.  Early wide dummy matmuls
warm the PE clock while the first DMA slices land.
"""

import numpy as np
import ml_dtypes

import concourse.bacc as bacc
import concourse.mybir as mybir
import concourse.tile as tile
from concourse.bass_utils import run_bass_kernel_spmd

B, P, DC, DN, H, DH = 4, 2048, 512, 64, 8, 64
NV = DC // DH
CH = 128          # positions per chunk
NCH = P // CH     # 16 chunks
NPR = NCH // 2    # 8 chunk pairs
HPC = 4           # heads per core
FH = HPC * DH     # 256 = stacked-head free width
FH2 = 2 * FH      # 512 = pair width
NCORES = 8
KC = DC // CH     # 4 feature K-chunks
NWARM = 11        # PE warm-up dummy matmuls

_BF16 = ml_dtypes.bfloat16

_cache = {}


def _softmax(x, axis=-1):
    e = np.exp(x - x.max(axis=axis, keepdims=True))
    return e / e.sum(axis=axis, keepdims=True)


def _build_program():
    nc = bacc.Bacc()
    f32 = mybir.dt.float32
    bf16 = mybir.dt.bfloat16
    mult = mybir.AluOpType.mult
    add = mybir.AluOpType.add

    w_d = nc.dram_tensor("w", [CH, KC, FH2], bf16, kind="ExternalInput")
    xct_d = nc.dram_tensor("xct", [4, CH, KC, P // 4], bf16, kind="ExternalInput")
    v_d = nc.dram_tensor("v", [CH, NCH * HPC], f32, kind="ExternalInput")
    trit_d = nc.dram_tensor("trit", [CH, CH], bf16, kind="ExternalInput")
    z_d = nc.dram_tensor("z", [CH, NCH * HPC], f32, kind="ExternalOutput")

    with tile.TileContext(nc) as tc:
        with (
            tc.tile_pool(name="persist", bufs=1) as pers,
            tc.tile_pool(name="work", bufs=3) as work,
            tc.tile_pool(name="mixp", bufs=3, space="PSUM") as mixp,
            tc.tile_pool(name="sp", bufs=2, space="PSUM") as sp,
            tc.tile_pool(name="psmall", bufs=1, space="PSUM") as psmall,
        ):
            w_sb = pers.tile([CH, KC, FH2], bf16, tag="w_sb")
            xcT = pers.tile([CH, 4, KC, P // 4], bf16, tag="xcT")
            v_sb = pers.tile([CH, NCH * HPC], f32, tag="v_sb")
            trit_sb = pers.tile([CH, CH], bf16, tag="trit_sb")
            vk_sb = pers.tile([CH, NCH, FH], bf16, tag="vk_sb")
            q_sb = pers.tile([CH, NCH, FH], bf16, tag="q_sb")
            z_sb = pers.tile([CH, NCH * HPC], f32, tag="z_sb")
            dumw = pers.tile([CH, FH2], bf16, tag="dumw")
            ones_sq = pers.tile([CH, CH], bf16, tag="ones_sq")
            # texw ping-pong: row 0 carries Tex[2j], rows 1-127 stay zero so
            # the all-ones stationary broadcasts row 0 (no K=1 matmul)
            texw_pp = [
                pers.tile([CH, FH2], bf16, name="texw0", tag="texw0"),
                pers.tile([CH, FH2], bf16, name="texw1", tag="texw1"),
            ]

            # ---- memsets + PE warm-up: wide dummy matmuls release the HAM
            # clock throttle while the DMA head is still in flight
            nc.gpsimd.memset(dumw[:], 0.0)
            nc.gpsimd.memset(ones_sq[:], 1.0)
            nc.gpsimd.memset(texw_pp[0][:], 0.0)
            nc.gpsimd.memset(texw_pp[1][:], 0.0)
            psum_dum = psmall.tile([CH, FH2], f32, tag="psum_dum")
            for i in range(NWARM):
                nc.tensor.matmul(
                    psum_dum[:], dumw[:, 0:CH], dumw[:], start=True, stop=True
                )

            # ---- loads: two HWDGE rings, 4KB per-partition lines.  Slice s
            # covers chunks {4s..4s+3} (pairs 2s, 2s+1).  sync ring: w (one
            # 512KB burst), v, trit, slices 1,3; scalar ring: slices 0,2.
            nc.scalar.dma_start(out=w_sb[:], in_=w_d[:])
            nc.sync.dma_start(out=xcT[:, 0], in_=xct_d[0])
            nc.sync.dma_start(out=v_sb[:], in_=v_d[:])
            nc.sync.dma_start(out=trit_sb[:], in_=trit_d[:])
            nc.scalar.dma_start(out=xcT[:, 1], in_=xct_d[1])
            nc.sync.dma_start(out=xcT[:, 2], in_=xct_d[2])
            nc.scalar.dma_start(out=xcT[:, 3], in_=xct_d[3])


            def emit_mix(c):
                psum_mix = mixp.tile([CH, FH2], f32, tag="psum_mix")
                for kc in range(KC):
                    nc.tensor.matmul(
                        psum_mix[:],
                        xcT[:, c // 4, kc, (c % 4) * CH : (c % 4 + 1) * CH],
                        w_sb[:, kc, :],
                        start=(kc == 0),
                        stop=(kc == KC - 1),
                    )
                # vk[p, i, h] = ktilde[p, i, h] * v[p, i]
                nc.vector.tensor_tensor(
                    out=vk_sb[:, c, :].rearrange("p (i h) -> p i h", h=DH),
                    in0=psum_mix[:, 0:FH].rearrange("p (i h) -> p i h", h=DH),
                    in1=v_sb[:, c * HPC : (c + 1) * HPC]
                    .unsqueeze(2)
                    .broadcast_to([CH, HPC, DH]),
                    op=mult,
                )
                nc.scalar.copy(q_sb[:, c, :], psum_mix[:, FH:FH2])

            def emit_pass2_mm(j):
                """S = trit @ vk_pair + carry (see module docstring).

                Pair 0 skips carry-A since Tex[0] = 0.
                """
                psum_S = sp.tile([CH, FH2], f32, tag="psum_S")
                nc.tensor.matmul(
                    psum_S[:],
                    trit_sb[:],
                    vk_sb[:, 2 * j : 2 * j + 2, :].rearrange("p c f -> p (c f)"),
                    start=True,
                    stop=False,
                )
                nc.tensor.matmul(
                    psum_S[:, FH:FH2],
                    ones_sq[:],
                    vk_sb[:, 2 * j, :],
                    start=False,
                    stop=(j == 0),
                )
                if j >= 1:
                    nc.tensor.matmul(
                        psum_S[:], ones_sq[:], texw_pp[j % 2][:], start=False,
                        stop=True,
                    )
                if j < NPR - 1:
                    # positions are chunk-reversed, so the full cumsum (the
                    # next pair's Tex) sits on partition 0 of the right half
                    nc.scalar.copy(
                        texw_pp[(j + 1) % 2][0:1, :].rearrange(
                            "a (c f) -> a c f", f=FH
                        ),
                        psum_S[0:1, FH:FH2].unsqueeze(1).broadcast_to([1, 2, FH]),
                    )
                return psum_S

            def emit_pass2_ve(j, psum_S, split=1):
                """ACT drain + DVE prod/reduce for pair j.

                split=2 pipelines the chain in half-pair chunks (used for the
                last pair, whose chain is the exposed tail).
                """
                for h in range(split):
                    f0, f1 = h * FH2 // split, (h + 1) * FH2 // split
                    s_sb = work.tile([CH, FH2 // split], bf16, tag=f"s_sb{split}")
                    nc.scalar.copy(s_sb[:], psum_S[:, f0:f1])
                    prod = work.tile([CH, FH2 // split], bf16, tag=f"prod{split}")
                    nc.vector.tensor_tensor(
                        out=prod[:],
                        in0=q_sb[:, 2 * j : 2 * j + 2, :].rearrange(
                            "p c f -> p (c f)"
                        )[:, f0:f1],
                        in1=s_sb[:],
                        op=mult,
                    )
                    nc.vector.tensor_reduce(
                        out=z_sb[
                            :,
                            2 * j * HPC + h * 2 * HPC // split : 2 * j * HPC
                            + (h + 1) * 2 * HPC // split,
                        ],
                        in_=prod[:].rearrange("p (ci h) -> p ci h", h=DH),
                        axis=mybir.AxisListType.X,
                        op=add,
                    )

            # ---- software-pipelined main stream, pass-2 lagging one pair
            prev_S = None
            for j in range(NPR):
                emit_mix(2 * j)
                if j >= 1:
                    prev_S = emit_pass2_mm(j - 1)
                emit_mix(2 * j + 1)
                if j >= 1:
                    emit_pass2_ve(j - 1, prev_S)
                if j == NPR - 1:
                    # overlap most of the z write-back with the last pair
                    nc.sync.dma_start(
                        out=z_d[:, 0 : 14 * HPC], in_=z_sb[:, 0 : 14 * HPC]
                    )
            prev_S = emit_pass2_mm(NPR - 1)
            # keep the PE (and its clock) busy while the last ve-chain drains
            for i in range(6):
                nc.tensor.matmul(
                    psum_dum[:], dumw[:, 0:CH], dumw[:], start=True, stop=True
                )
            emit_pass2_ve(NPR - 1, prev_S, split=2)

            nc.sync.dma_start(
                out=z_d[:, 14 * HPC : NCH * HPC], in_=z_sb[:, 14 * HPC : NCH * HPC]
            )

    nc.finalize()
    return nc


def _host_inputs(x_cat, x_num, W_K, W_Q, W_pred, W_V):
    """Per-core input maps. Core c = batch (c//2), head-group (c%2)."""
    pk = _softmax(W_K.astype(np.float64)).astype(np.float32)
    pq = _softmax(W_Q.astype(np.float64)).astype(np.float32)
    pp = _softmax(W_pred.astype(np.float64)).astype(np.float32)
    pv = _softmax(W_V.astype(np.float64)).astype(np.float32)

    # positions are stored chunk-reversed (r = 127 - p%128) so the inclusive
    # cumsum matmul leaves the full chunk sum on partition 0
    trit = np.tril(np.ones((CH, CH), np.float32))
    perm = np.arange(P).reshape(NCH, CH)[:, ::-1].reshape(P)
    eye = np.eye(DH, dtype=np.float32)
    v_full = np.einsum("bpd,id->bpi", x_num, pv)  # [B, P, H] fp32, host-side

    in_maps = []
    for core in range(NCORES):
        b, hg = core // 2, core % 2
        heads = range(hg * HPC, (hg + 1) * HPC)
        W = np.zeros((DC, FH2), np.float32)
        for j, i in enumerate(heads):
            # ktilde cols: W[(v,g), j*64+h] = pk[i,v] * pp[i,h,g]
            W[:, j * DH : (j + 1) * DH] = (
                pk[i][:, None, None] * pp[i].T[None, :, :]
            ).reshape(DC, DH)
            # xq cols: W[(v,h), FH + j*64+h'] = pq[i,v] * delta(h,h')
            W[:, FH + j * DH : FH + (j + 1) * DH] = np.kron(pq[i][:, None], eye)
        # per-partition contiguous slice blocks (4KB lines), chunk-reversed
        xq8 = x_cat[b][perm].T.reshape(KC, CH, 4, P // 4).transpose(2, 1, 0, 3)
        wq = W.reshape(KC, CH, FH2).transpose(1, 0, 2)
        # v in device layout [p, (chunk, head)]
        v_core = v_full[b][perm][:, hg * HPC : (hg + 1) * HPC]  # [P, HPC]
        v_dev = np.ascontiguousarray(
            v_core.reshape(NCH, CH, HPC).transpose(1, 0, 2).reshape(CH, NCH * HPC)
        )
        in_maps.append(
            {
                "xct": np.ascontiguousarray(xq8).astype(_BF16),
                "w": np.ascontiguousarray(wq).astype(_BF16),
                "v": v_dev,
                "trit": trit.astype(_BF16),
            }
        )
    return in_maps


def _run(inputs, **spmd_kwargs):
    if "nc" not in _cache:
        _cache["nc"] = _build_program()
    nc = _cache["nc"]

    in_maps = _host_inputs(**inputs)
    res = run_bass_kernel_spmd(nc, in_maps, list(range(NCORES)), **spmd_kwargs)

    perm = np.arange(P).reshape(NCH, CH)[:, ::-1].reshape(P)
    out = np.zeros((B, P, H), np.float32)
    for core in range(NCORES):
        b, hg = core // 2, core % 2
        z = res.results[core]["z"]  # [128, NCH*HPC]
        z = z.reshape(CH, NCH, HPC).transpose(1, 0, 2).reshape(P, HPC)
        out[b, :, hg * HPC : (hg + 1) * HPC] = z[perm]
    return out, res


def kernel(x_cat, x_num, W_K, W_Q, W_pred, W_V):
    out, _ = _run(
        dict(x_cat=x_cat, x_num=x_num, W_K=W_K, W_Q=W_Q, W_pred=W_pred, W_V=W_V)
    )
    return out
